# revision 9
# baseline (speedup 1.0000x reference)
"""Trainium2 Bass kernel for nn_CausalAttention (N=4096, 8 heads, DH=32).

v2: head-parallel across 8 NeuronCores (1 head per core).

  - Scores on alternating PE row-tiles (tile_position rows 0/32) via
    2-way row-packed kT and 4x-replicated qT, so consecutive score
    matmuls overlap fill/drain in the PE array.
  - bf16 qT/kT score operands (fast LDWEIGHTS).
  - K-bias dropped (softmax-invariant); Q-bias folded into the Q
    PSUM->SBUF conversion; V-bias folded into v_all (weighted by P
    whose rows sum to 1; row q=0 has P==0 so output stays 0).
  - exp split ACT (true exp) / DVE (Schraudolph bit-trick tensor_scalar:
    int16(s*C1 + C2) bitcast bf16; ~40% of tiles, ~+0.4% rel err).
  - Strict-causal masking: post-exp 0/1 triangle multiply on GpSimd for
    the 4 diagonal k-tiles per q-block; fully-masked column prefixes
    skipped in exp AND PV (no memsets).
  - Normalization: colsum (ones column in v_all) -> [8,64] reshape DMA
    -> reciprocal -> broadcast DMA -> one tensor_mul.
  - PSUM: 3x [128,1024] score tiles + 2x [33,512] PV accumulators.
"""

import math

import numpy as np
import ml_dtypes

import concourse.bass as bass
import concourse.mybir as mybir
from concourse import bacc
from concourse.tile import TileContext
from concourse.bass_utils import run_bass_kernel_spmd

# Problem constants (hardcoded per harness contract).
B, CQ, CK, CH, NH, H, W = 1, 256, 256, 256, 8, 64, 64
DH = CH // NH            # 32
N = H * W                # 4096
QB = 512                 # queries per block
NQB = N // QB            # 8
KT = 128                 # keys per k-tile
SCALE = 1.0 / math.sqrt(DH)

# Schraudolph exp constants for bf16 bit patterns: bits = int(x*C1 + C2)
SCH_C1 = 128.0 * math.log2(math.e) * SCALE
SCH_C2 = 128.0 * 127 - 7.41 + 0.5

F32 = mybir.dt.float32
F32R = mybir.dt.float32r
BF16 = mybir.dt.bfloat16
I16 = mybir.dt.int16

# exp routing per pair-group: A=ACT true exp, D=DVE schraudolph.
ROUTE = "AADAADAADA"  # A:7 D:3 per 10

ALU = mybir.AluOpType

_CACHED_NC = None


def _build():
    nc = bacc.Bacc("TRN2", target_bir_lowering=False, debug=False, num_devices=1)

    qin_d = nc.dram_tensor("qin", [CQ, N], F32, kind="ExternalInput")
    kin_d = nc.dram_tensor("kin", [CK, N], F32, kind="ExternalInput")
    wq_d = nc.dram_tensor("wqt", [CQ, 128], F32, kind="ExternalInput")
    wk_d = nc.dram_tensor("wkt", [CK, 128], F32, kind="ExternalInput")
    wv_d = nc.dram_tensor("wvt", [CK, DH], F32, kind="ExternalInput")
    bq_d = nc.dram_tensor("bqr", [128, 1], F32, kind="ExternalInput")
    bv_d = nc.dram_tensor("bvr", [128, DH], F32, kind="ExternalInput")
    out_d = nc.dram_tensor("out", [DH, N], F32, kind="ExternalOutput")

    # strict-causal triangle for the diagonal window: tm[kk, t] = 1.0 iff kk < t
    tm_np = (np.arange(128)[:, None] < np.arange(128)[None, :]).astype(
        ml_dtypes.bfloat16
    )
    tm_d = nc.inline_tensor(tm_np, name="tmask")

    with TileContext(nc) as tc:
        with (
            tc.tile_pool(name="constp", bufs=1) as constp,
            tc.tile_pool(name="bigp", bufs=1) as bigp,
            tc.tile_pool(name="workp", bufs=4) as workp,
            tc.tile_pool(name="spool", bufs=3, space="PSUM") as spool,
            tc.tile_pool(name="opool", bufs=1, space="PSUM") as opool,
            tc.tile_pool(name="pvpool", bufs=1, space="PSUM") as pvpool,
        ):
            # ---- DMAs: weights + first input slices first ----
            kin_sb = bigp.tile([128, 2, N], F32R, name="kin_sb")
            qin_sb = bigp.tile([128, 2, N], F32R, name="qin_sb")
            kin_ap = kin_d.ap().rearrange("(c p) n -> p c n", p=128).bitcast(F32R)
            qin_ap = qin_d.ap().rearrange("(c p) n -> p c n", p=128).bitcast(F32R)

            wk_sb = constp.tile([128, 2, 128], F32R, name="wk_sb")
            nc.sync.dma_start(
                wk_sb[:], wk_d.ap().rearrange("(c p) m -> p c m", p=128).bitcast(F32R)
            )
            wq_sb = constp.tile([128, 2, 128], F32R, name="wq_sb")
            nc.sync.dma_start(
                wq_sb[:], wq_d.ap().rearrange("(c p) m -> p c m", p=128).bitcast(F32R)
            )
            sl0 = slice(0, QB)
            nc.sync.dma_start(kin_sb[:, :, sl0], kin_ap[:, :, sl0])
            nc.scalar.dma_start(qin_sb[:, :, sl0], qin_ap[:, :, sl0])
            wv_sb = constp.tile([128, 2, DH], F32R, name="wv_sb")
            nc.sync.dma_start(
                wv_sb[:], wv_d.ap().rearrange("(c p) m -> p c m", p=128).bitcast(F32R)
            )
            bq_sb = constp.tile([128, 1], F32, name="bq_sb")
            nc.sync.dma_start(bq_sb[:], bq_d.ap())
            bv_sb = constp.tile([128, DH], F32, name="bv_sb")
            nc.sync.dma_start(bv_sb[:], bv_d.ap())
            tm_sb = constp.tile([128, 128], BF16, name="tm_sb")
            nc.sync.dma_start(tm_sb[:], tm_d.ap())
            for s in range(1, 8):
                sl = slice(QB * s, QB * (s + 1))
                nc.sync.dma_start(kin_sb[:, :, sl], kin_ap[:, :, sl])
                nc.scalar.dma_start(qin_sb[:, :, sl], qin_ap[:, :, sl])

            # ---- persistent SBUF operands ----
            qT = bigp.tile([128, N], BF16, name="qT")   # q + bq, 4x replicated
            # kT[32*(j%2)+d, 128*(j//2)+kk] = k^T[d, 128j+kk]   (no bias)
            kT = bigp.tile([64, N], BF16, name="kT")
            v_all = bigp.tile([128, N // KT, 34], BF16, name="v_all")  # v+bv | ones
            nc.vector.memset(v_all[:, :, DH : DH + 1], 1.0)

            stage_q = []     # deferred tail stages, advanced one per group
            route_i = [0]    # exp routing counter
            obank = opool.tile([128, QB], F32, name="obank", tag="o")

            def emit_proj(s):
                """Q/K/V projections for input slice s (512 positions)."""
                ksl = slice(QB * s, QB * (s + 1))
                pj = spool.tile([128, 1024], F32, name="pj", tag="s")
                for ch in range(2):
                    nc.tensor.matmul(
                        pj[:, 0:QB],
                        wq_sb[:, ch, :],
                        qin_sb[:, ch, ksl],
                        start=(ch == 0),
                        stop=(ch == 1),
                    )
                for ch in range(2):
                    nc.tensor.matmul(
                        pj[:, QB : 2 * QB],
                        wk_sb[:, ch, :],
                        kin_sb[:, ch, ksl],
                        start=(ch == 0),
                        stop=(ch == 1),
                    )
                nc.vector.tensor_scalar_add(qT[:, ksl], pj[:, 0:QB], bq_sb[:])
                # kT 2-way pack: k-tile j=4s+ci (ci=0..3): u=ci%2, g=2s+ci//2.
                # half u: src cols QB+128u and QB+128u+256 -> dst cols 256s+.
                for u in range(2):
                    psl = slice(32 * u, 32 * u + 32)
                    ksrc = pj[psl, QB + 128 * u : QB + 128 * u + 384].rearrange(
                        "p (a c) -> p a c", a=3
                    )[:, 0:3:2, :]
                    kdst = kT[psl, 256 * s : 256 * s + 256].rearrange(
                        "p (a c) -> p a c", a=2
                    )
                    nc.scalar.activation(
                        kdst, ksrc, mybir.ActivationFunctionType.Copy
                    )
                pv = pvpool.tile([128, 128], F32, name="pv", tag="v")
                for t in range(4):
                    nsl = slice(QB * s + KT * t, QB * s + KT * (t + 1))
                    for ch in range(2):
                        nc.tensor.matmul(
                            pv[:, DH * t : DH * (t + 1)],
                            kin_sb[:, ch, nsl],
                            wv_sb[:, ch, :],
                            start=(ch == 0),
                            stop=(ch == 1),
                        )
                # v_all[:, 4s+t, 0:DH] = pv[:, 32t:32t+32] + bv  (one op)
                src = pv[:].rearrange("p (t d) -> p t d", t=4)
                nc.vector.tensor_tensor(
                    v_all[:, 4 * s : 4 * s + 4, 0:DH],
                    src,
                    bv_sb[:].unsqueeze(1).broadcast_to([128, 4, DH]),
                    op=ALU.add,
                )

            # ---- attention tails (staged to decouple from the PE stream) ----
            def tail_a(st):
                cs8 = workp.tile([8, 64], F32, name="cs8")
                cs = workp.tile([1, QB], F32, name="cs")
                nc.vector.tensor_scalar_add(cs[:], st["o_ps"][DH : DH + 1, :], 1e-30)
                nc.sync.dma_start(cs8[:], cs[:])
                st["cs8"] = cs8

            def tail_b(st):
                cr8 = workp.tile([8, 64], F32, name="cr8")
                nc.vector.reciprocal(cr8[:], st["cs8"][:])
                st["cr8"] = cr8

            def tail_c(st):
                crl = workp.tile([1, QB], F32, name="crl")
                nc.sync.dma_start(crl[:], st["cr8"][:])
                st["crl"] = crl

            def tail_d(st):
                rep = workp.tile([DH, QB], F32, name="rep")
                src = st["crl"][:].unsqueeze(1).broadcast_to([1, DH, QB])
                nc.sync.dma_start(rep[:], src)
                st["rep"] = rep

            def tail_e(st):
                qb = st["qb"]
                out_sb = workp.tile([DH, QB], F32, name="out_sb")
                nc.vector.tensor_mul(out_sb[:], st["o_ps"][0:DH, :], st["rep"][:])
                nc.sync.dma_start(out_d.ap()[:, QB * qb : QB * (qb + 1)], out_sb[:])

            pends = []

            def flush_pv(pend):
                pqb, pnkt, po_ps, g, p_sb = pend
                for u in range(2):
                    j = 2 * g + u
                    o = max(0, KT * j - QB * pqb)
                    nc.tensor.matmul(
                        po_ps[:, o:QB],
                        v_all[:, j, 0 : DH + 1],
                        p_sb[:, QB * u + o : QB * (u + 1)],
                        start=(j == 0),
                        stop=(j == pnkt - 1),
                        skip_group_check=True,
                    )
                if 2 * g + 1 == pnkt - 1:  # last pair of this q-block
                    st = {"qb": pqb, "o_ps": po_ps}
                    stage_q.append(lambda st=st: tail_a(st))
                    stage_q.append(lambda st=st: tail_b(st))
                    stage_q.append(lambda st=st: tail_c(st))
                    stage_q.append(lambda st=st: tail_d(st))
                    stage_q.append(lambda st=st: tail_e(st))

            def emit_attn(qb):
                nkt = 4 * (qb + 1)
                npair = nkt // 2
                ob = 64 * (qb % 2)
                o_ps = obank[ob : ob + DH + 1, :]

                for g in range(npair):
                    diag = g >= npair - 2
                    s_ps = spool.tile([128, 1024], F32, name="s_ps", tag="s")
                    for u in range(2):
                        j = 2 * g + u
                        ju, jg = j % 2, j // 2
                        o = max(0, KT * j - QB * qb)
                        nc.tensor.matmul(
                            s_ps[:, QB * u + o : QB * (u + 1)],
                            kT[32 * ju : 32 * ju + 32, KT * jg : KT * (jg + 1)],
                            qT[32 * ju : 32 * ju + 32, QB * qb + o : QB * (qb + 1)],
                            start=True,
                            stop=True,
                        )
                    p_sb = workp.tile([128, 1024], BF16, name="p_sb", bufs=10)
                    r = ROUTE[route_i[0] % len(ROUTE)]
                    route_i[0] += 1
                    if not diag:
                        if r == "A":
                            nc.scalar.activation(
                                p_sb[:],
                                s_ps[:],
                                mybir.ActivationFunctionType.Exp,
                                scale=SCALE,
                            )
                        else:
                            nc.vector.tensor_scalar(
                                p_sb[:].bitcast(I16),
                                s_ps[:],
                                SCH_C1,
                                SCH_C2,
                                op0=ALU.mult,
                                op1=ALU.add,
                            )
                    else:
                        for u in range(2):
                            j = 2 * g + u
                            o = max(0, KT * j - QB * qb)
                            psl = slice(QB * u + o, QB * (u + 1))
                            if r == "A":
                                nc.scalar.activation(
                                    p_sb[:, psl],
                                    s_ps[:, psl],
                                    mybir.ActivationFunctionType.Exp,
                                    scale=SCALE,
                                )
                            else:
                                nc.vector.tensor_scalar(
                                    p_sb[:, psl].bitcast(I16),
                                    s_ps[:, psl],
                                    SCH_C1,
                                    SCH_C2,
                                    op0=ALU.mult,
                                    op1=ALU.add,
                                )
                        # strict-causal triangle on both 128-wide windows
                        for u in range(2):
                            j = 2 * g + u
                            o = KT * j - QB * qb
                            wsl = slice(QB * u + o, QB * u + o + KT)
                            nc.gpsimd.tensor_mul(
                                p_sb[:, wsl], p_sb[:, wsl], tm_sb[:]
                            )
                    pends.append((qb, nkt, o_ps, g, p_sb))
                    if len(pends) > 4:
                        flush_pv(pends.pop(0))
                    if stage_q:
                        stage_q.pop(0)()
                if qb == NQB - 1:
                    while pends:
                        flush_pv(pends.pop(0))

            emit_proj(0)
            emit_proj(1)
            for qb in range(NQB):
                if qb + 2 < NQB:
                    emit_proj(qb + 2)
                emit_attn(qb)
            while stage_q:
                stage_q.pop(0)()

    nc.finalize()
    return nc


def _get_nc():
    global _CACHED_NC
    if _CACHED_NC is None:
        _CACHED_NC = _build()
    return _CACHED_NC


def _prep_in_maps(inputs):
    f = lambda a: np.ascontiguousarray(np.asarray(a, dtype=np.float32))
    query = f(inputs["query"]).reshape(CQ, N)
    key_feat = f(inputs["key_feat"]).reshape(CK, N)

    def wnorm(v, g):
        v = f(v)
        g = f(g)
        return g[:, None] * v / np.linalg.norm(v, axis=1, keepdims=True)

    wq = wnorm(inputs["vq"], inputs["gq"])
    wk = wnorm(inputs["vk"], inputs["gk"])
    wv = wnorm(inputs["vv"], inputs["gv"])
    bq, bv = f(inputs["bq"]), f(inputs["bv"])

    in_maps = []
    for c in range(NH):
        rows = slice(DH * c, DH * (c + 1))
        in_maps.append(
            {
                "qin": query,
                "kin": key_feat,
                "wqt": np.ascontiguousarray(np.tile(wq[rows].T, (1, 4))),
                "wkt": np.ascontiguousarray(np.tile(wk[rows].T, (1, 4))),
                "wvt": np.ascontiguousarray(wv[rows].T),
                "bqr": np.ascontiguousarray(np.tile(bq[rows], 4)[:, None]),
                "bvr": np.ascontiguousarray(np.tile(bv[rows][None, :], (128, 1))),
            }
        )
    return in_maps


def _run(inputs, trace=False, **kwargs):
    nc = _get_nc()
    in_maps = _prep_in_maps(inputs)
    res = None
    for attempt in range(3):
        try:
            res = run_bass_kernel_spmd(
                nc, in_maps, core_ids=list(range(NH)), trace=trace, **kwargs
            )
            break
        except Exception:
            if attempt == 2:
                raise

    out = np.empty((B, CH, H, W), dtype=np.float32)
    for c in range(NH):
        oc = res.results[c]["out"]  # [DH, N] (O^T layout)
        out[0, DH * c : DH * (c + 1)] = oc.reshape(DH, H, W)
    return out, res


def kernel(**inputs) -> np.ndarray:
    out, _ = _run(inputs, trace=False)
    return out


# revision 10
# speedup vs baseline: 1.2839x; 1.2839x over previous
"""Trainium2 Bass kernel for nn_CausalAttention (N=4096, 8 heads, DH=32).

v2: head-parallel across 8 NeuronCores (1 head per core).

  - Scores on alternating PE row-tiles (tile_position rows 0/32) via
    2-way row-packed kT and 4x-replicated qT, so consecutive score
    matmuls overlap fill/drain in the PE array.
  - bf16 qT/kT score operands (fast LDWEIGHTS).
  - K-bias dropped (softmax-invariant); Q-bias folded into the Q
    PSUM->SBUF conversion; V-bias folded into v_all (weighted by P
    whose rows sum to 1; row q=0 has P==0 so output stays 0).
  - exp split ACT (true exp) / DVE (Schraudolph bit-trick tensor_scalar:
    int16(s*C1 + C2) bitcast bf16; ~40% of tiles, ~+0.4% rel err).
  - Strict-causal masking: post-exp 0/1 triangle multiply on GpSimd for
    the 4 diagonal k-tiles per q-block; fully-masked column prefixes
    skipped in exp AND PV (no memsets).
  - Normalization: colsum (ones column in v_all) -> [8,64] reshape DMA
    -> reciprocal -> broadcast DMA -> one tensor_mul.
  - PSUM: 3x [128,1024] score tiles + 2x [33,512] PV accumulators.
"""

import math

import numpy as np
import ml_dtypes

import concourse.bass as bass
import concourse.mybir as mybir
from concourse import bacc
from concourse.tile import TileContext
from concourse.bass_utils import run_bass_kernel_spmd

# Problem constants (hardcoded per harness contract).
B, CQ, CK, CH, NH, H, W = 1, 256, 256, 256, 8, 64, 64
DH = CH // NH            # 32
N = H * W                # 4096
QB = 512                 # queries per block
NQB = N // QB            # 8
KT = 128                 # keys per k-tile
SCALE = 1.0 / math.sqrt(DH)

# Schraudolph exp constants for bf16 bit patterns: bits = int(x*C1 + C2)
SCH_C1 = 128.0 * math.log2(math.e) * SCALE
SCH_C2 = 128.0 * 127 - 7.41 + 0.5

F32 = mybir.dt.float32
F32R = mybir.dt.float32r
BF16 = mybir.dt.bfloat16
I16 = mybir.dt.int16

# exp routing per pair-group: A=ACT true exp, D=DVE schraudolph.
ROUTE = "AADAADAADA"  # A:7 D:3 per 10

ALU = mybir.AluOpType

_CACHED_NC = None


def _build():
    nc = bacc.Bacc("TRN2", target_bir_lowering=False, debug=False, num_devices=1)

    qin_d = nc.dram_tensor("qin", [CQ, N], F32, kind="ExternalInput")
    kin_d = nc.dram_tensor("kin", [CK, N], F32, kind="ExternalInput")
    wq_d = nc.dram_tensor("wqt", [CQ, 128], F32, kind="ExternalInput")
    wk_d = nc.dram_tensor("wkt", [CK, 128], F32, kind="ExternalInput")
    wv_d = nc.dram_tensor("wvt", [CK, DH], F32, kind="ExternalInput")
    bq_d = nc.dram_tensor("bqr", [128, 1], F32, kind="ExternalInput")
    bv_d = nc.dram_tensor("bvr", [128, DH], F32, kind="ExternalInput")
    out_d = nc.dram_tensor("out", [DH, N], F32, kind="ExternalOutput")

    # strict-causal triangle for the diagonal window: tm[kk, t] = 1.0 iff kk < t
    tm_np = (np.arange(128)[:, None] < np.arange(128)[None, :]).astype(
        ml_dtypes.bfloat16
    )
    tm_d = nc.inline_tensor(tm_np, name="tmask")

    with TileContext(nc) as tc:
        with (
            tc.tile_pool(name="constp", bufs=1) as constp,
            tc.tile_pool(name="bigp", bufs=1) as bigp,
            tc.tile_pool(name="workp", bufs=4) as workp,
            tc.tile_pool(name="spool", bufs=3, space="PSUM") as spool,
            tc.tile_pool(name="opool", bufs=2, space="PSUM") as opool,
        ):
            # ---- DMAs: weights + first input slices first ----
            kin_sb = bigp.tile([128, 2, N], F32R, name="kin_sb")
            qin_sb = bigp.tile([128, 2, N], F32R, name="qin_sb")
            kin_ap = kin_d.ap().rearrange("(c p) n -> p c n", p=128).bitcast(F32R)
            qin_ap = qin_d.ap().rearrange("(c p) n -> p c n", p=128).bitcast(F32R)

            wk_sb = constp.tile([128, 2, 128], F32R, name="wk_sb")
            nc.sync.dma_start(
                wk_sb[:], wk_d.ap().rearrange("(c p) m -> p c m", p=128).bitcast(F32R)
            )
            wq_sb = constp.tile([128, 2, 128], F32R, name="wq_sb")
            nc.sync.dma_start(
                wq_sb[:], wq_d.ap().rearrange("(c p) m -> p c m", p=128).bitcast(F32R)
            )
            sl0 = slice(0, QB)
            nc.sync.dma_start(kin_sb[:, :, sl0], kin_ap[:, :, sl0])
            nc.scalar.dma_start(qin_sb[:, :, sl0], qin_ap[:, :, sl0])
            wv_sb = constp.tile([128, 2, DH], F32R, name="wv_sb")
            nc.sync.dma_start(
                wv_sb[:], wv_d.ap().rearrange("(c p) m -> p c m", p=128).bitcast(F32R)
            )
            bq_sb = constp.tile([128, 1], F32, name="bq_sb")
            nc.sync.dma_start(bq_sb[:], bq_d.ap())
            bv_sb = constp.tile([128, DH], F32, name="bv_sb")
            nc.sync.dma_start(bv_sb[:], bv_d.ap())
            tm_sb = constp.tile([128, 128], BF16, name="tm_sb")
            nc.sync.dma_start(tm_sb[:], tm_d.ap())
            for s in range(1, 8):
                sl = slice(QB * s, QB * (s + 1))
                nc.sync.dma_start(kin_sb[:, :, sl], kin_ap[:, :, sl])
                nc.scalar.dma_start(qin_sb[:, :, sl], qin_ap[:, :, sl])

            # ---- persistent SBUF operands ----
            qT = bigp.tile([128, N], BF16, name="qT")   # q + bq, 4x replicated
            # kT[32*(j%2)+d, 128*(j//2)+kk] = k^T[d, 128j+kk]   (no bias)
            kT = bigp.tile([64, N], BF16, name="kT")
            v_all = bigp.tile([128, N // KT, 34], BF16, name="v_all")  # v+bv | ones
            nc.vector.memset(v_all[:, :, DH : DH + 1], 1.0)

            stage_q = []     # deferred tail stages, advanced one per group
            route_i = [0]    # exp routing counter


            def emit_proj(s):
                """Q/K/V projections for input slice s (512 positions)."""
                ksl = slice(QB * s, QB * (s + 1))
                pj = spool.tile([128, 1024], F32, name="pj", tag="s")
                for ch in range(2):
                    nc.tensor.matmul(
                        pj[:, 0:QB],
                        wq_sb[:, ch, :],
                        qin_sb[:, ch, ksl],
                        start=(ch == 0),
                        stop=(ch == 1),
                    )
                for ch in range(2):
                    nc.tensor.matmul(
                        pj[:, QB : 2 * QB],
                        wk_sb[:, ch, :],
                        kin_sb[:, ch, ksl],
                        start=(ch == 0),
                        stop=(ch == 1),
                    )
                nc.vector.tensor_scalar_add(qT[:, ksl], pj[:, 0:QB], bq_sb[:])
                # kT 2-way pack: k-tile j=4s+ci (ci=0..3): u=ci%2, g=2s+ci//2.
                # half u: src cols QB+128u and QB+128u+256 -> dst cols 256s+.
                for u in range(2):
                    psl = slice(32 * u, 32 * u + 32)
                    ksrc = pj[psl, QB + 128 * u : QB + 128 * u + 384].rearrange(
                        "p (a c) -> p a c", a=3
                    )[:, 0:3:2, :]
                    kdst = kT[psl, 256 * s : 256 * s + 256].rearrange(
                        "p (a c) -> p a c", a=2
                    )
                    nc.scalar.activation(
                        kdst, ksrc, mybir.ActivationFunctionType.Copy
                    )
                pv = opool.tile([128, 128], F32, name="pv", tag="o")
                for t in range(4):
                    nsl = slice(QB * s + KT * t, QB * s + KT * (t + 1))
                    for ch in range(2):
                        nc.tensor.matmul(
                            pv[:, DH * t : DH * (t + 1)],
                            kin_sb[:, ch, nsl],
                            wv_sb[:, ch, :],
                            start=(ch == 0),
                            stop=(ch == 1),
                        )
                # v_all[:, 4s+t, 0:DH] = pv[:, 32t:32t+32] + bv  (one op)
                src = pv[:].rearrange("p (t d) -> p t d", t=4)
                nc.vector.tensor_tensor(
                    v_all[:, 4 * s : 4 * s + 4, 0:DH],
                    src,
                    bv_sb[:].unsqueeze(1).broadcast_to([128, 4, DH]),
                    op=ALU.add,
                )

            # ---- attention tails (staged to decouple from the PE stream) ----
            def tail_a(st):
                cs8 = workp.tile([8, 64], F32, name="cs8")
                cs = workp.tile([1, QB], F32, name="cs")
                nc.vector.tensor_scalar_add(cs[:], st["o_ps"][DH : DH + 1, :], 1e-30)
                nc.sync.dma_start(cs8[:], cs[:])
                st["cs8"] = cs8

            def tail_b(st):
                cr8 = workp.tile([8, 64], F32, name="cr8")
                nc.vector.reciprocal(cr8[:], st["cs8"][:])
                st["cr8"] = cr8

            def tail_c(st):
                crl = workp.tile([1, QB], F32, name="crl")
                nc.sync.dma_start(crl[:], st["cr8"][:])
                st["crl"] = crl

            def tail_d(st):
                rep = workp.tile([DH, QB], F32, name="rep")
                src = st["crl"][:].unsqueeze(1).broadcast_to([1, DH, QB])
                nc.sync.dma_start(rep[:], src)
                st["rep"] = rep

            def tail_e(st):
                qb = st["qb"]
                out_sb = workp.tile([DH, QB], F32, name="out_sb")
                nc.vector.tensor_mul(out_sb[:], st["o_ps"][0:DH, :], st["rep"][:])
                nc.sync.dma_start(out_d.ap()[:, QB * qb : QB * (qb + 1)], out_sb[:])

            pends = []

            def flush_pv(pend):
                pqb, pnkt, po_ps, g, p_sb = pend
                for u in range(2):
                    j = 2 * g + u
                    o = max(0, KT * j - QB * pqb)
                    nc.tensor.matmul(
                        po_ps[:, o:QB],
                        v_all[:, j, 0 : DH + 1],
                        p_sb[:, QB * u + o : QB * (u + 1)],
                        start=(j == 0),
                        stop=(j == pnkt - 1),
                        skip_group_check=True,
                    )
                if 2 * g + 1 == pnkt - 1:  # last pair of this q-block
                    st = {"qb": pqb, "o_ps": po_ps}
                    stage_q.append(lambda st=st: tail_a(st))
                    stage_q.append(lambda st=st: tail_b(st))
                    stage_q.append(lambda st=st: tail_c(st))
                    stage_q.append(lambda st=st: tail_d(st))
                    stage_q.append(lambda st=st: tail_e(st))

            def emit_attn(qb):
                nkt = 4 * (qb + 1)
                npair = nkt // 2
                o_ps = opool.tile([DH + 1, QB], F32, name="o_ps", tag="o")

                for g in range(npair):
                    diag = g >= npair - 2
                    s_ps = spool.tile([128, 1024], F32, name="s_ps", tag="s")
                    for u in range(2):
                        j = 2 * g + u
                        ju, jg = j % 2, j // 2
                        o = max(0, KT * j - QB * qb)
                        nc.tensor.matmul(
                            s_ps[:, QB * u + o : QB * (u + 1)],
                            kT[32 * ju : 32 * ju + 32, KT * jg : KT * (jg + 1)],
                            qT[32 * ju : 32 * ju + 32, QB * qb + o : QB * (qb + 1)],
                            start=True,
                            stop=True,
                        )
                    p_sb = workp.tile([128, 1024], BF16, name="p_sb", bufs=10)
                    r = ROUTE[route_i[0] % len(ROUTE)]
                    route_i[0] += 1
                    if not diag:
                        if r == "A":
                            nc.scalar.activation(
                                p_sb[:],
                                s_ps[:],
                                mybir.ActivationFunctionType.Exp,
                                scale=SCALE,
                            )
                        else:
                            nc.vector.tensor_scalar(
                                p_sb[:].bitcast(I16),
                                s_ps[:],
                                SCH_C1,
                                SCH_C2,
                                op0=ALU.mult,
                                op1=ALU.add,
                            )
                    else:
                        for u in range(2):
                            j = 2 * g + u
                            o = max(0, KT * j - QB * qb)
                            psl = slice(QB * u + o, QB * (u + 1))
                            if r == "A":
                                nc.scalar.activation(
                                    p_sb[:, psl],
                                    s_ps[:, psl],
                                    mybir.ActivationFunctionType.Exp,
                                    scale=SCALE,
                                )
                            else:
                                nc.vector.tensor_scalar(
                                    p_sb[:, psl].bitcast(I16),
                                    s_ps[:, psl],
                                    SCH_C1,
                                    SCH_C2,
                                    op0=ALU.mult,
                                    op1=ALU.add,
                                )
                        # strict-causal triangle on both 128-wide windows
                        for u in range(2):
                            j = 2 * g + u
                            o = KT * j - QB * qb
                            wsl = slice(QB * u + o, QB * u + o + KT)
                            nc.gpsimd.tensor_mul(
                                p_sb[:, wsl], p_sb[:, wsl], tm_sb[:]
                            )
                    pends.append((qb, nkt, o_ps, g, p_sb))
                    if len(pends) > 4:
                        flush_pv(pends.pop(0))
                    if stage_q:
                        stage_q.pop(0)()
                if qb == NQB - 1:
                    while pends:
                        flush_pv(pends.pop(0))

            emit_proj(0)
            emit_proj(1)
            for qb in range(NQB):
                if qb + 2 < NQB:
                    emit_proj(qb + 2)
                emit_attn(qb)
            while stage_q:
                stage_q.pop(0)()

    nc.finalize()
    return nc


def _get_nc():
    global _CACHED_NC
    if _CACHED_NC is None:
        _CACHED_NC = _build()
    return _CACHED_NC


def _prep_in_maps(inputs):
    f = lambda a: np.ascontiguousarray(np.asarray(a, dtype=np.float32))
    query = f(inputs["query"]).reshape(CQ, N)
    key_feat = f(inputs["key_feat"]).reshape(CK, N)

    def wnorm(v, g):
        v = f(v)
        g = f(g)
        return g[:, None] * v / np.linalg.norm(v, axis=1, keepdims=True)

    wq = wnorm(inputs["vq"], inputs["gq"])
    wk = wnorm(inputs["vk"], inputs["gk"])
    wv = wnorm(inputs["vv"], inputs["gv"])
    bq, bv = f(inputs["bq"]), f(inputs["bv"])

    in_maps = []
    for c in range(NH):
        rows = slice(DH * c, DH * (c + 1))
        in_maps.append(
            {
                "qin": query,
                "kin": key_feat,
                "wqt": np.ascontiguousarray(np.tile(wq[rows].T, (1, 4))),
                "wkt": np.ascontiguousarray(np.tile(wk[rows].T, (1, 4))),
                "wvt": np.ascontiguousarray(wv[rows].T),
                "bqr": np.ascontiguousarray(np.tile(bq[rows], 4)[:, None]),
                "bvr": np.ascontiguousarray(np.tile(bv[rows][None, :], (128, 1))),
            }
        )
    return in_maps


def _run(inputs, trace=False, **kwargs):
    nc = _get_nc()
    in_maps = _prep_in_maps(inputs)
    res = None
    for attempt in range(3):
        try:
            res = run_bass_kernel_spmd(
                nc, in_maps, core_ids=list(range(NH)), trace=trace, **kwargs
            )
            break
        except Exception:
            if attempt == 2:
                raise

    out = np.empty((B, CH, H, W), dtype=np.float32)
    for c in range(NH):
        oc = res.results[c]["out"]  # [DH, N] (O^T layout)
        out[0, DH * c : DH * (c + 1)] = oc.reshape(DH, H, W)
    return out, res


def kernel(**inputs) -> np.ndarray:
    out, _ = _run(inputs, trace=False)
    return out
